# revision 1
# baseline (speedup 1.0000x reference)
"""Trainium2 Bass kernel for the BoundaryCorrectionModule problem.

Full inputs in, full output out. Internally: pure data-parallel over the
batch dim across 8 NeuronCores (2048 rows each). Activations live in
SBUF in feature-major ("transposed") layout [128p, 8kc, batch], bf16.
Weights are host-transposed and packed so every device DMA is a single
contiguous read. All concat-matmuls are decomposed into 1024x1024 units
accumulated in PSUM (f32); ACT evacuates PSUM with fused bias +
sigmoid/tanh; DVE applies the gating / GRU arithmetic.

Step-0 algebraic fold: S0 = 0.5*(h_prev+h_next) = 0.5*(M+D), so step 0
uses W_SM' = W_SM + 0.5*W_SS and W_SD' = W_SD + 0.5*W_SS and never
materializes S0 (saves one full D x D unit-GEMM per step-0).
"""

import numpy as np
import ml_dtypes

import concourse.bass as bass
import concourse.mybir as mybir
import concourse.tile as tile
from concourse import bacc
from concourse.bass_utils import run_bass_kernel_spmd

BF16 = ml_dtypes.bfloat16
F32 = np.float32

B = 16384          # full batch
D = 1024           # feature dim
NCORES = 8
BC = B // NCORES   # per-core batch (2048)
NHALF = 2          # batch sub-passes per core (weights re-streamed per pass)
H = BC // NHALF    # rows per pass (1024)
FREE = 512         # matmul moving free dim (ISA cap; one PSUM bank of f32)
NB = H // FREE     # moving tiles per pass
KC = D // 128      # contraction chunks (8)
MC = D // 128      # output-feature chunks (8)
NSTEP = 2

# unit ids (each is a [D, D] block of some weight, pre-transposed to
# lhsT layout on the host)
U_SS, U_SM, U_SD = 0, 1, 2
U_gS_M, U_gS_D = 3, 4
U_Mz_S, U_Mz_M, U_Mz_D = 5, 6, 7
U_Mr_S, U_Mr_M, U_Mr_D = 8, 9, 10
U_Mh_S, U_Mh_rM, U_Mh_D = 11, 12, 13
U_DS, U_DM, U_DD = 14, 15, 16
U_gD_S, U_gD_M = 17, 18
U_SM0, U_SD0 = 19, 20      # step-0 folded: W_SM + 0.5 W_SS, W_SD + 0.5 W_SS
NUNITS = 21

# bias ids
B_gS, B_Mz, B_Mr, B_Mh, B_gD = 0, 1, 2, 3, 4
NBIAS = 5

SIG = mybir.ActivationFunctionType.Sigmoid
TANH = mybir.ActivationFunctionType.Tanh

_BUILD_CACHE = {}


def _pack_unit(wt_block):
    """[D(k), D(m)] f32 (already W.T) -> [MC, 128, KC, 128] bf16 contiguous,
    laid out exactly as the SBUF weight tile [p, kc, m] per mc."""
    w = wt_block.reshape(KC, 128, MC, 128)          # (kc, p, mc, m)
    w = np.transpose(w, (2, 1, 0, 3))               # (mc, p, kc, m)
    return np.ascontiguousarray(w.astype(BF16))


def _pack_acts(hT_core):
    """[D, BC] f32 (feature-major slice for one core) ->
    [NHALF, 128, KC, H] bf16 contiguous (per-pass SBUF tile layout)."""
    x = hT_core.reshape(KC, 128, BC)                # (kc, p, b)
    x = np.transpose(x, (1, 0, 2))                  # (p, kc, b)
    x = x.reshape(128, KC, NHALF, NB, FREE)
    x = np.transpose(x, (2, 3, 0, 1, 4))            # (h, n, p, kc, b)
    return np.ascontiguousarray(x.astype(BF16))


def _build():
    """Build + compile the Bass module once per process."""
    key = (BC, FREE)
    if key in _BUILD_CACHE:
        return _BUILD_CACHE[key]

    nc = bacc.Bacc("TRN2", target_bir_lowering=False, debug=False)
    bf = mybir.dt.bfloat16
    f32 = mybir.dt.float32

    hp_d = nc.dram_tensor("hp", [NHALF, NB, 128, KC, FREE], bf, kind="ExternalInput")
    hn_d = nc.dram_tensor("hn", [NHALF, NB, 128, KC, FREE], bf, kind="ExternalInput")
    wu_d = nc.dram_tensor("wu", [NUNITS, MC, 128, KC, 128], bf, kind="ExternalInput")
    bias_d = nc.dram_tensor("bias", [128, NBIAS, MC], f32, kind="ExternalInput")
    rs_d = nc.dram_tensor("rs", [128, 1], f32, kind="ExternalInput")
    out_d = nc.dram_tensor("out", [NHALF, MC, 128, H], f32, kind="ExternalOutput")

    with tile.TileContext(nc) as tc:
        with (
            tc.tile_pool(name="const", bufs=1) as const_p,
            tc.tile_pool(name="st", bufs=2) as st_p,
            tc.tile_pool(name="aux", bufs=1) as aux_p,
            tc.tile_pool(name="wp", bufs=12) as w_p,
            tc.tile_pool(name="tp", bufs=6) as t_p,
            tc.tile_pool(name="dp", bufs=4) as d_p,
            tc.tile_pool(name="op", bufs=4) as o_p,
            tc.tile_pool(name="ps", bufs=8, space="PSUM") as ps_p,
        ):
            bias_t = const_p.tile([128, NBIAS, MC], f32)
            nc.gpsimd.dma_start(bias_t[:], bias_d.ap()[:, :, :])
            rs_t = const_p.tile([128, 1], f32)
            nc.gpsimd.dma_start(rs_t[:], rs_d.ap()[:, :])

            def load_w(u, mc):
                w = w_p.tile([128, KC, 128], bf, tag="w", name=f"w{u}_{mc}")
                nc.sync.dma_start(w[:], wu_d.ap()[u, mc])
                return w

            # Every activation tensor is NB n-half tiles of [128, KC, FREE]:
            # Tile's dependency tracking is per-tile, so halving the tile
            # makes cross-phase deps finer (earlier starts, tighter pipeline).
            def new_state(pool, tag, name):
                return tuple(
                    pool.tile([128, KC, FREE], bf, tag=f"{tag}{n}",
                              name=f"{name}_{n}", uniquify=True)
                    for n in range(NB))

            def phase(units, bias_idx, evac, preloaded=None):
                """One matmul phase over the full pass batch.

                units: list of (unit_id, src_state); accumulated in PSUM.
                evac(psum, mc, n, bias_ap) consumes each PSUM sub-tile.
                """
                for mc in range(MC):
                    wts = []
                    for (u, src) in units:
                        if preloaded and (u, mc) in preloaded:
                            w = preloaded[(u, mc)]
                        else:
                            w = load_w(u, mc)
                        wts.append((w, src))
                    psums = []
                    for n in range(NB):
                        p = ps_p.tile([128, FREE], f32, tag="p", name=f"p{mc}_{n}")
                        psums.append(p)
                    total = len(units) * KC
                    i = 0
                    for (w, src) in wts:
                        for kc in range(KC):
                            for n in range(NB):
                                inst = nc.tensor.matmul(
                                    psums[n][:, :],
                                    w[:, kc, :],
                                    src[n][:, kc, :],
                                    start=(i == 0),
                                    stop=(i == total - 1),
                                )
                                if n > 0:
                                    # Same stationary operand as the previous
                                    # matmul: skip the redundant LDWEIGHTS.
                                    inst.ins.ldweights = False
                            i += 1
                    b_ap = bias_t[:, bias_idx, mc:mc + 1]
                    for n in range(NB):
                        evac(psums[n], mc, n, b_ap)

            def evac_plain(dst, func):
                def f(psum, mc, n, b_ap):
                    nc.scalar.activation(
                        dst[n][:, mc, :], psum[:, :], func, bias=b_ap)
                return f

            def evac_gated(dst, func, gate):
                def f(psum, mc, n, b_ap):
                    t = t_p.tile([128, FREE], bf, tag="t", name=f"t{mc}_{n}")
                    nc.scalar.activation(t[:], psum[:, :], func, bias=b_ap)
                    nc.vector.tensor_mul(
                        dst[n][:, mc, :], t[:], gate[n][:, mc, :])
                return f

            def evac_gru(dst, M_old, z):
                def f(psum, mc, n, b_ap):
                    t = t_p.tile([128, FREE], bf, tag="t", name=f"t{mc}_{n}")
                    nc.scalar.activation(t[:], psum[:, :], TANH, bias=b_ap)
                    d = d_p.tile([128, FREE], bf, tag="d", name=f"d{mc}_{n}")
                    nc.vector.tensor_sub(d[:], t[:], M_old[n][:, mc, :])
                    nc.vector.tensor_mul(d[:], d[:], z[n][:, mc, :])
                    nc.vector.tensor_add(dst[n][:, mc, :], M_old[n][:, mc, :], d[:])
                return f

            for h in range(NHALF):
                M = new_state(st_p, "M", f"M_{h}")
                Dv = new_state(st_p, "D", f"D_{h}")
                if h == 0:
                    # Get the first phase's mc=0 weights onto the (FIFO) DMA
                    # ring first, then the M halves, so PE can start early.
                    pre = {(U_gS_M, 0): load_w(U_gS_M, 0),
                           (U_gS_D, 0): load_w(U_gS_D, 0)}
                else:
                    pre = None
                for n in range(NB):
                    nc.sync.dma_start(M[n][:], hp_d.ap()[h, n])
                for n in range(NB):
                    nc.sync.dma_start(Dv[n][:], hn_d.ap()[h, n])
                S = None

                for step in range(NSTEP):
                    last = step == NSTEP - 1

                    GS = new_state(aux_p, "GS", f"GS_{h}_{step}")
                    phase([(U_gS_M, M), (U_gS_D, Dv)], B_gS,
                          evac_plain(GS, SIG), preloaded=pre)
                    pre = None

                    S_new = new_state(st_p, "S", f"Sn_{h}_{step}")
                    if step == 0:
                        s_units = [(U_SM0, M), (U_SD0, Dv)]
                    else:
                        s_units = [(U_SS, S), (U_SM, M), (U_SD, Dv)]
                    phase(s_units, B_gS, evac_gated(S_new, TANH, GS))

                    z = new_state(aux_p, "z", f"z_{h}_{step}")
                    phase([(U_Mz_S, S_new), (U_Mz_M, M), (U_Mz_D, Dv)], B_Mz,
                          evac_plain(z, SIG))

                    rM = new_state(aux_p, "rM", f"rM_{h}_{step}")
                    phase([(U_Mr_S, S_new), (U_Mr_M, M), (U_Mr_D, Dv)], B_Mr,
                          evac_gated(rM, SIG, M))

                    M_new = new_state(st_p, "M", f"Mn_{h}_{step}")
                    phase([(U_Mh_S, S_new), (U_Mh_rM, rM), (U_Mh_D, Dv)], B_Mh,
                          evac_gru(M_new, M, z))

                    GD = new_state(aux_p, "GD", f"GD_{h}_{step}")
                    phase([(U_gD_S, S_new), (U_gD_M, M_new)], B_gD,
                          evac_plain(GD, SIG))

                    d_units = [(U_DS, S_new), (U_DM, M_new), (U_DD, Dv)]
                    if not last:
                        D_new = new_state(st_p, "D", f"Dn_{h}_{step}")
                        phase(d_units, B_gD, evac_gated(D_new, TANH, GD))
                        S, M, Dv = S_new, M_new, D_new
                    else:
                        # Fused tail: D_new = tanh(.)*GD exists only per-chunk;
                        # out = M_new + rs*(S_new + D_new) streams straight out.
                        def evac_final(psum, mc, n, b_ap,
                                       _S=S_new, _M=M_new, _GD=GD, _h=h):
                            t = t_p.tile([128, FREE], bf, tag="t",
                                         name=f"t{mc}_{n}")
                            nc.scalar.activation(t[:], psum[:, :], TANH, bias=b_ap)
                            d = d_p.tile([128, FREE], bf, tag="d",
                                         name=f"d{mc}_{n}")
                            nc.vector.tensor_mul(d[:], t[:], _GD[n][:, mc, :])
                            o = o_p.tile([128, FREE], f32, tag="o",
                                         name=f"o_{_h}_{mc}_{n}")
                            nc.vector.tensor_add(o[:], _S[n][:, mc, :], d[:])
                            nc.vector.tensor_scalar_mul(o[:], o[:], rs_t[:, 0:1])
                            nc.vector.tensor_add(o[:], o[:], _M[n][:, mc, :])
                            nc.sync.dma_start(
                                out_d.ap()[_h, mc, :, bass.ts(n, FREE)], o[:])
                        phase(d_units, B_gD, evac_final)

    nc.compile()
    _BUILD_CACHE[key] = nc
    return nc


def _pack_inputs(h_prev, h_next, W_SS, W_SM, W_SD, W_Mz, b_Mz, W_Mr, b_Mr,
                 W_Mh, b_Mh, W_DS, W_DM, W_DD, W_gS, b_gS, W_gD, b_gD,
                 residual_scale):
    """Host-side packing: transposes, bf16 casts, per-core sharding."""
    units = [None] * NUNITS
    f = np.float32

    def T(w):
        return np.ascontiguousarray(np.asarray(w, f).T)

    t_ss, t_sm, t_sd = T(W_SS), T(W_SM), T(W_SD)
    units[U_SS] = _pack_unit(t_ss)
    units[U_SM] = _pack_unit(t_sm)
    units[U_SD] = _pack_unit(t_sd)
    units[U_SM0] = _pack_unit(t_sm + f(0.5) * t_ss)
    units[U_SD0] = _pack_unit(t_sd + f(0.5) * t_ss)
    gs = T(W_gS)                       # [2D, D]
    units[U_gS_M] = _pack_unit(gs[:D])
    units[U_gS_D] = _pack_unit(gs[D:])
    for base, Wx in ((U_Mz_S, W_Mz), (U_Mr_S, W_Mr), (U_Mh_S, W_Mh)):
        wx = T(Wx)                     # [3D, D]
        units[base] = _pack_unit(wx[:D])
        units[base + 1] = _pack_unit(wx[D:2 * D])
        units[base + 2] = _pack_unit(wx[2 * D:])
    units[U_DS] = _pack_unit(T(W_DS))
    units[U_DM] = _pack_unit(T(W_DM))
    units[U_DD] = _pack_unit(T(W_DD))
    gd = T(W_gD)
    units[U_gD_S] = _pack_unit(gd[:D])
    units[U_gD_M] = _pack_unit(gd[D:])
    wu = np.stack(units)               # [NUNITS, MC, 128, KC, 128] bf16

    bias = np.stack([np.asarray(b, f) for b in (b_gS, b_Mz, b_Mr, b_Mh, b_gD)])
    bias = bias.reshape(NBIAS, MC, 128)
    bias = np.ascontiguousarray(np.transpose(bias, (2, 0, 1)))  # [128, NBIAS, MC]

    rs = np.full((128, 1), np.asarray(residual_scale, f), dtype=f)

    hpT = np.asarray(h_prev, f).T      # [D, B] view
    hnT = np.asarray(h_next, f).T

    in_maps = []
    for c in range(NCORES):
        sl = slice(c * BC, (c + 1) * BC)
        in_maps.append({
            "hp": _pack_acts(np.ascontiguousarray(hpT[:, sl])),
            "hn": _pack_acts(np.ascontiguousarray(hnT[:, sl])),
            "wu": wu,
            "bias": bias,
            "rs": rs,
        })
    return in_maps


def _unpack_output(results):
    """Per-core [NHALF, MC, 128, H] f32 -> full [B, D] f32."""
    blocks = []
    for c in range(NCORES):
        a = results[c]["out"]                       # [NHALF, MC, 128, H]
        a = np.transpose(a, (1, 2, 0, 3)).reshape(D, BC)  # feature-major
        blocks.append(a)
    outT = np.concatenate(blocks, axis=1)           # [D, B]
    return np.ascontiguousarray(outT.T)


def run(trace=False, tmpdir=None, trace_kwargs=None, **inputs):
    """Extended entry point: returns (output, BassKernelResults)."""
    nc = _build()
    in_maps = _pack_inputs(**inputs)
    res = run_bass_kernel_spmd(
        nc, in_maps, core_ids=list(range(NCORES)),
        trace=trace, tmpdir=tmpdir, **(trace_kwargs or {}))
    return _unpack_output(res.results), res


def kernel(**inputs):
    # Grading entry point: never trace (a stray BASS_TRACE env would route
    # run_bass_kernel_spmd into the NTFF-hook path, which needs extra setup).
    import os
    os.environ["BASS_NEVER_TRACE"] = "1"
    try:
        out, _ = run(**inputs)
    finally:
        os.environ.pop("BASS_NEVER_TRACE", None)
    return out



# revision 8
# speedup vs baseline: 1.4172x; 1.4172x over previous
"""Trainium2 Bass kernel for the BoundaryCorrectionModule problem.

Full inputs in, full output out. Pure data-parallel over the batch dim
across 8 NeuronCores (2048 rows each). Activations live in SBUF in
feature-major layout [128p, kc, batch]. Matmuls run per-unit in either
fp8e4 (DoubleRow perf mode, 2 K-chunks per instruction) or bf16; both
accumulate into the same per-phase PSUM group because every weight unit
is pre-scaled by a per-phase power-of-2 (exact in bf16) and the ACT
evacuation compensates with a runtime per-phase scale.

State dtype policy (driven by UNIT_DTYPE): bf16 "master" copies feed the
DVE elementwise math; fp8 copies are made only for fp8 matmul operands.

Step-0 algebraic fold: S0 = 0.5*(h_prev+h_next), so step 0 uses
W_SM' = W_SM + 0.5*W_SS and W_SD' = W_SD + 0.5*W_SS.
"""

import numpy as np
import ml_dtypes

import concourse.bass as bass
import concourse.mybir as mybir
import concourse.tile as tile
from concourse import bacc
from concourse.bass_utils import run_bass_kernel_spmd

BF16 = ml_dtypes.bfloat16
E4M3 = ml_dtypes.float8_e4m3
F32 = np.float32

B = 16384          # full batch
D = 1024           # feature dim
NCORES = 8
BC = B // NCORES   # per-core batch (2048)
NHALF = 2          # batch sub-passes per core
H = BC // NHALF    # rows per pass (1024)
FREE = 512         # matmul moving free dim (one PSUM bank of f32)
NB = H // FREE     # moving tiles per pass
KC = D // 128      # contraction chunks (8)
KP = KC // 2       # DoubleRow chunk-pairs (4)
MC = D // 128      # output-feature chunks (8)
NSTEP = 2

# unit ids ([D, D] blocks, pre-transposed to lhsT layout on the host)
U_SS, U_SM, U_SD = 0, 1, 2
U_gS_M, U_gS_D = 3, 4
U_Mz_S, U_Mz_M, U_Mz_D = 5, 6, 7
U_Mr_S, U_Mr_M, U_Mr_D = 8, 9, 10
U_Mh_S, U_Mh_rM, U_Mh_D = 11, 12, 13
U_DS, U_DM, U_DD = 14, 15, 16
U_gD_S, U_gD_M = 17, 18
U_SM0, U_SD0 = 19, 20      # step-0 folded
NUNITS = 21

B_gS, B_Mz, B_Mr, B_Mh, B_gD = 0, 1, 2, 3, 4
NBIAS = 5

# phase-instance scale slots (per (step,phase) ACT compensation scale)
PH_NAMES = ["gS", "S", "z", "r", "h", "gD", "D"]
NPH = 2 * len(PH_NAMES)


def ph_slot(step, ph):
    return step * len(PH_NAMES) + PH_NAMES.index(ph)


# units per phase instance (host scale computation mirrors the builder)
PH_UNITS = {
    (0, "gS"): [U_gS_M, U_gS_D], (1, "gS"): [U_gS_M, U_gS_D],
    (0, "S"): [U_SM0, U_SD0], (1, "S"): [U_SS, U_SM, U_SD],
    (0, "z"): [U_Mz_S, U_Mz_M, U_Mz_D], (1, "z"): [U_Mz_S, U_Mz_M, U_Mz_D],
    (0, "r"): [U_Mr_S, U_Mr_M, U_Mr_D], (1, "r"): [U_Mr_S, U_Mr_M, U_Mr_D],
    (0, "h"): [U_Mh_S, U_Mh_rM, U_Mh_D], (1, "h"): [U_Mh_S, U_Mh_rM, U_Mh_D],
    (0, "gD"): [U_gD_S, U_gD_M], (1, "gD"): [U_gD_S, U_gD_M],
    (0, "D"): [U_DS, U_DM, U_DD], (1, "D"): [U_DS, U_DM, U_DD],
}

# ---------------------------------------------------------------------------
# Per-unit matmul dtype config: (step, phase, src) -> 8 or 16 (default 8).
# src tags: 'M' (carry state), 'D' (Dv), 'S' (S_new), 'rM', 'Mn' (M_new).
# Chosen from the numpy error sim (sim2/sim3): the 14 units with the largest
# per-unit err^2 contribution stay bf16; predicted final rel err 0.0164.
UNIT_DTYPE = {
    (0, "D", "D"): 16, (0, "D", "Mn"): 16,
    (0, "S", "D"): 16, (0, "S", "M"): 16,
    (0, "h", "D"): 16, (0, "h", "S"): 16, (0, "h", "rM"): 16,
    (0, "z", "D"): 16, (0, "z", "M"): 16,
    (1, "S", "M"): 16,
    (1, "h", "D"): 16, (1, "h", "S"): 16, (1, "h", "rM"): 16,
    (1, "z", "M"): 16,
}


def udt(step, ph, src):
    return UNIT_DTYPE.get((step, ph, src), 8)


def _consumers_M(step):
    return [(step, "gS", "M"), (step, "S", "M"), (step, "z", "M"),
            (step, "r", "M")]


def _consumers_Dv(step):
    return [(step, "gS", "D"), (step, "S", "D"), (step, "z", "D"),
            (step, "r", "D"), (step, "h", "D"), (step, "D", "D")]


def _consumers_Snew(step):
    c = [(step, "z", "S"), (step, "r", "S"), (step, "h", "S"),
         (step, "gD", "S"), (step, "D", "S")]
    if step == 0:
        c.append((1, "S", "S"))
    return c


def _consumers_Mnew(step):
    c = [(step, "gD", "Mn"), (step, "D", "Mn")]
    if step == 0:
        c += _consumers_M(1)
    return c


def _needs(consumers, d):
    return any(udt(*c) == d for c in consumers)


SIG = mybir.ActivationFunctionType.Sigmoid
TANH = mybir.ActivationFunctionType.Tanh
DR = mybir.MatmulPerfMode.DoubleRow

_BUILD_CACHE = {}


def _pack_unit(wt_block, dtype):
    """[D(k), D(m)] f32 (already W.T, pre-scaled) -> [MC, 128, KC, 128]."""
    w = wt_block.reshape(KC, 128, MC, 128)          # (kc, p, mc, m)
    w = np.transpose(w, (2, 1, 0, 3))               # (mc, p, kc, m)
    return np.ascontiguousarray(w.astype(dtype))


def _pack_acts(hT_core, dtype):
    """[D, BC] f32 -> [NHALF, NB, 128, KC, FREE] (per-pass SBUF layout)."""
    x = hT_core.reshape(KC, 128, BC)                # (kc, p, b)
    x = np.transpose(x, (1, 0, 2))                  # (p, kc, b)
    x = x.reshape(128, KC, NHALF, NB, FREE)
    x = np.transpose(x, (2, 3, 0, 1, 4))            # (h, n, p, kc, b)
    return np.ascontiguousarray(x.astype(dtype))


def _cfg_key():
    return tuple(sorted(UNIT_DTYPE.items()))


def _build():
    key = _cfg_key()
    if key in _BUILD_CACHE:
        return _BUILD_CACHE[key]

    nc = bacc.Bacc("TRN2", target_bir_lowering=False, debug=False)
    bf = mybir.dt.bfloat16
    f8 = mybir.dt.float8e4
    f32 = mybir.dt.float32

    hp16_d = nc.dram_tensor("hp16", [NHALF, NB, 128, KC, FREE], bf,
                            kind="ExternalInput")
    hp8_d = nc.dram_tensor("hp8", [NHALF, NB, 128, KC, FREE], f8,
                           kind="ExternalInput")
    hn16_d = nc.dram_tensor("hn16", [NHALF, NB, 128, KC, FREE], bf,
                            kind="ExternalInput")
    hn8_d = nc.dram_tensor("hn8", [NHALF, NB, 128, KC, FREE], f8,
                           kind="ExternalInput")
    wu8_d = nc.dram_tensor("wu8", [NUNITS, MC, 128, KC, 128], f8,
                           kind="ExternalInput")
    wu16_d = nc.dram_tensor("wu16", [NUNITS, MC, 128, KC, 128], bf,
                            kind="ExternalInput")
    bias_d = nc.dram_tensor("bias", [128, NBIAS, MC], f32, kind="ExternalInput")
    scl_d = nc.dram_tensor("scl", [128, NPH], f32, kind="ExternalInput")
    rs_d = nc.dram_tensor("rs", [128, 1], f32, kind="ExternalInput")
    out_d = nc.dram_tensor("out", [NHALF, MC, 128, H], f32,
                           kind="ExternalOutput")

    with tile.TileContext(nc) as tc:
        with (
            tc.tile_pool(name="const", bufs=1) as const_p,
            tc.tile_pool(name="st", bufs=2) as st_p,
            tc.tile_pool(name="aux", bufs=1) as aux_p,
            tc.tile_pool(name="wp8", bufs=8) as w8_p,
            tc.tile_pool(name="wp16", bufs=6) as w16_p,
            tc.tile_pool(name="tp", bufs=4) as t_p,
            tc.tile_pool(name="dp", bufs=3) as d_p,
            tc.tile_pool(name="op", bufs=2) as o_p,
            tc.tile_pool(name="ps", bufs=8, space="PSUM") as ps_p,
        ):
            bias_t = const_p.tile([128, NBIAS, MC], f32)
            nc.gpsimd.dma_start(bias_t[:], bias_d.ap()[:, :, :])
            scl_t = const_p.tile([128, NPH], f32)
            nc.gpsimd.dma_start(scl_t[:], scl_d.ap()[:, :])
            rs_t = const_p.tile([128, 1], f32)
            nc.gpsimd.dma_start(rs_t[:], rs_d.ap()[:, :])

            def load_w(u, mc, d):
                if d == 8:
                    w = w8_p.tile([128, KC, 128], f8, tag="w8",
                                  name=f"w8_{u}_{mc}")
                    nc.sync.dma_start(w[:], wu8_d.ap()[u, mc])
                else:
                    w = w16_p.tile([128, KC, 128], bf, tag="w16",
                                   name=f"w16_{u}_{mc}")
                    nc.sync.dma_start(w[:], wu16_d.ap()[u, mc])
                return w

            def new_state(pool, tag, name, dtype):
                return tuple(
                    pool.tile([128, KC, FREE], dtype, tag=f"{tag}{n}",
                              name=f"{name}_{n}", uniquify=True)
                    for n in range(NB))

            def mk_state(pool, tag, name, need16, need8):
                """State handle: dict dtype->tiles (or None)."""
                s = {16: None, 8: None}
                if need16:
                    s[16] = new_state(pool, tag + "f", name + "f", bf)
                if need8:
                    s[8] = new_state(pool, tag + "q", name + "q", f8)
                return s

            def phase(step, ph, units, bias_idx, evac, preloaded=None):
                """units: list of (unit_id, state_dict, src_tag)."""
                scl_ap = scl_t[:, ph_slot(step, ph):ph_slot(step, ph) + 1]
                for mc in range(MC):
                    wts = []
                    for (u, st, src) in units:
                        d = udt(step, ph, src)
                        w = None
                        if preloaded:
                            w = preloaded.get((u, mc))
                        if w is None:
                            w = load_w(u, mc, d)
                        wts.append((w, st[d], d))
                    psums = []
                    for n in range(NB):
                        p = ps_p.tile([128, FREE], f32, tag="p", name=f"p{mc}_{n}")
                        psums.append(p)
                    total = sum(KP if d == 8 else KC for (_, _, d) in wts)
                    i = 0
                    for (w, src, d) in wts:
                        if d == 8:
                            for kp in range(KP):
                                for n in range(NB):
                                    inst = nc.tensor.matmul(
                                        psums[n][:, :],
                                        w[:, 2 * kp:2 * kp + 2, :],
                                        src[n][:, 2 * kp:2 * kp + 2, :],
                                        start=(i == 0),
                                        stop=(i == total - 1),
                                        perf_mode=DR,
                                    )
                                    if n > 0:
                                        inst.ins.ldweights = False
                                i += 1
                        else:
                            for kc in range(KC):
                                for n in range(NB):
                                    inst = nc.tensor.matmul(
                                        psums[n][:, :],
                                        w[:, kc, :],
                                        src[n][:, kc, :],
                                        start=(i == 0),
                                        stop=(i == total - 1),
                                    )
                                    if n > 0:
                                        inst.ins.ldweights = False
                                i += 1
                    b_ap = bias_t[:, bias_idx, mc:mc + 1]
                    for n in range(NB):
                        evac(psums[n], mc, n, b_ap, scl_ap)

            def evac_plain(dst, func):
                # dst: bf16 tiles (gates)
                def fn(psum, mc, n, b_ap, scl_ap):
                    nc.scalar.activation(
                        dst[n][:, mc, :], psum[:, :], func,
                        bias=b_ap, scale=scl_ap)
                return fn

            def evac_gated(dst, func, gate):
                """dst state dict; writes 16 (and copies to 8) or 8 direct."""
                def fn(psum, mc, n, b_ap, scl_ap):
                    t = t_p.tile([128, FREE], bf, tag="t", name=f"t{mc}_{n}")
                    nc.scalar.activation(t[:], psum[:, :], func,
                                         bias=b_ap, scale=scl_ap)
                    if dst[16] is not None:
                        nc.vector.tensor_mul(
                            dst[16][n][:, mc, :], t[:], gate[n][:, mc, :])
                        if dst[8] is not None:
                            nc.gpsimd.tensor_copy(
                                dst[8][n][:, mc, :], dst[16][n][:, mc, :])
                    else:
                        nc.vector.tensor_mul(
                            dst[8][n][:, mc, :], t[:], gate[n][:, mc, :])
                return fn

            def evac_gru(dst, M_old, z):
                def fn(psum, mc, n, b_ap, scl_ap):
                    t = t_p.tile([128, FREE], bf, tag="t", name=f"t{mc}_{n}")
                    nc.scalar.activation(t[:], psum[:, :], TANH,
                                         bias=b_ap, scale=scl_ap)
                    d = d_p.tile([128, FREE], bf, tag="d", name=f"d{mc}_{n}")
                    nc.vector.tensor_sub(d[:], t[:], M_old[16][n][:, mc, :])
                    nc.vector.tensor_mul(d[:], d[:], z[n][:, mc, :])
                    nc.vector.tensor_add(
                        dst[16][n][:, mc, :], M_old[16][n][:, mc, :], d[:])
                    if dst[8] is not None:
                        nc.gpsimd.tensor_copy(
                            dst[8][n][:, mc, :], dst[16][n][:, mc, :])
                return fn

            for h in range(NHALF):
                need_M8 = _needs(_consumers_M(0), 8)
                need_D8 = _needs(_consumers_Dv(0), 8)
                need_D16 = _needs(_consumers_Dv(0), 16)
                M = mk_state(st_p, "M", f"M_{h}", True, need_M8)
                Dv = mk_state(st_p, "D", f"D_{h}", need_D16, need_D8)
                if h == 0:
                    d0 = udt(0, "gS", "M")
                    d1 = udt(0, "gS", "D")
                    pre = {(U_gS_M, 0): load_w(U_gS_M, 0, d0),
                           (U_gS_D, 0): load_w(U_gS_D, 0, d1)}
                else:
                    pre = None
                for n in range(NB):
                    nc.sync.dma_start(M[16][n][:], hp16_d.ap()[h, n])
                if need_M8:
                    for n in range(NB):
                        nc.sync.dma_start(M[8][n][:], hp8_d.ap()[h, n])
                if need_D16:
                    for n in range(NB):
                        nc.sync.dma_start(Dv[16][n][:], hn16_d.ap()[h, n])
                if need_D8:
                    for n in range(NB):
                        nc.sync.dma_start(Dv[8][n][:], hn8_d.ap()[h, n])
                S = None

                for step in range(NSTEP):
                    last = step == NSTEP - 1

                    # GS and z share an aux ring (non-overlapping lifetimes),
                    # as do rM and GD.
                    GS = new_state(aux_p, "g1", f"GS_{h}_{step}", bf)
                    phase(step, "gS", [(U_gS_M, M, "M"), (U_gS_D, Dv, "D")],
                          B_gS, evac_plain(GS, SIG), preloaded=pre)
                    pre = None

                    sc = _consumers_Snew(step)
                    S_new = mk_state(st_p, "S", f"Sn_{h}_{step}",
                                     True, _needs(sc, 8))
                    if step == 0:
                        s_units = [(U_SM0, M, "M"), (U_SD0, Dv, "D")]
                    else:
                        s_units = [(U_SS, S, "S"), (U_SM, M, "M"),
                                   (U_SD, Dv, "D")]
                    phase(step, "S", s_units, B_gS, evac_gated(S_new, TANH, GS))

                    z = new_state(aux_p, "g1", f"z_{h}_{step}", bf)
                    phase(step, "z", [(U_Mz_S, S_new, "S"), (U_Mz_M, M, "M"),
                                      (U_Mz_D, Dv, "D")],
                          B_Mz, evac_plain(z, SIG))

                    rd = udt(step, "h", "rM")
                    rM = mk_state(aux_p, "g2", f"rM_{h}_{step}",
                                  rd == 16, rd == 8)
                    phase(step, "r", [(U_Mr_S, S_new, "S"), (U_Mr_M, M, "M"),
                                      (U_Mr_D, Dv, "D")],
                          B_Mr, evac_gated(rM, SIG, M[16]))

                    mnc = _consumers_Mnew(step)
                    M_new = mk_state(st_p, "M", f"Mn_{h}_{step}",
                                     True, _needs(mnc, 8))
                    phase(step, "h", [(U_Mh_S, S_new, "S"), (U_Mh_rM, rM, "rM"),
                                      (U_Mh_D, Dv, "D")],
                          B_Mh, evac_gru(M_new, M, z))

                    GD = new_state(aux_p, "g2f", f"GD_{h}_{step}", bf)
                    phase(step, "gD", [(U_gD_S, S_new, "S"),
                                       (U_gD_M, M_new, "Mn")],
                          B_gD, evac_plain(GD, SIG))

                    d_units = [(U_DS, S_new, "S"), (U_DM, M_new, "Mn"),
                               (U_DD, Dv, "D")]
                    if not last:
                        dc = _consumers_Dv(1)
                        D_new = mk_state(st_p, "D", f"Dn_{h}_{step}",
                                         _needs(dc, 16), _needs(dc, 8))
                        phase(step, "D", d_units, B_gD,
                              evac_gated(D_new, TANH, GD))
                        S, M, Dv = S_new, M_new, D_new
                    else:
                        # Fused tail: out = M_new + rs*(S_new + D_new)
                        def evac_final(psum, mc, n, b_ap, scl_ap,
                                       _S=S_new, _M=M_new, _GD=GD, _h=h):
                            t = t_p.tile([128, FREE], bf, tag="t",
                                         name=f"t{mc}_{n}")
                            nc.scalar.activation(t[:], psum[:, :], TANH,
                                                 bias=b_ap, scale=scl_ap)
                            d = d_p.tile([128, FREE], bf, tag="d",
                                         name=f"d{mc}_{n}")
                            nc.vector.tensor_mul(d[:], t[:], _GD[n][:, mc, :])
                            o = o_p.tile([128, FREE], f32, tag="o",
                                         name=f"o_{_h}_{mc}_{n}")
                            nc.vector.tensor_add(o[:], _S[16][n][:, mc, :], d[:])
                            nc.vector.tensor_scalar_mul(o[:], o[:], rs_t[:, 0:1])
                            nc.vector.tensor_add(o[:], o[:], _M[16][n][:, mc, :])
                            nc.sync.dma_start(
                                out_d.ap()[_h, mc, :, bass.ts(n, FREE)], o[:])
                        phase(step, "D", d_units, B_gD, evac_final)

    nc.compile()
    _BUILD_CACHE[key] = nc
    return nc


def _pack_inputs(h_prev, h_next, W_SS, W_SM, W_SD, W_Mz, b_Mz, W_Mr, b_Mr,
                 W_Mh, b_Mh, W_DS, W_DM, W_DD, W_gS, b_gS, W_gD, b_gD,
                 residual_scale):
    """Host-side packing: transposes, per-phase scaling, casts, sharding."""
    f = np.float32

    def T(w):
        return np.ascontiguousarray(np.asarray(w, f).T)

    t_ss, t_sm, t_sd = T(W_SS), T(W_SM), T(W_SD)
    gs = T(W_gS)
    gd = T(W_gD)
    mz, mr, mh = T(W_Mz), T(W_Mr), T(W_Mh)
    raw = {
        U_SS: t_ss, U_SM: t_sm, U_SD: t_sd,
        U_SM0: t_sm + f(0.5) * t_ss, U_SD0: t_sd + f(0.5) * t_ss,
        U_gS_M: gs[:D], U_gS_D: gs[D:],
        U_Mz_S: mz[:D], U_Mz_M: mz[D:2 * D], U_Mz_D: mz[2 * D:],
        U_Mr_S: mr[:D], U_Mr_M: mr[D:2 * D], U_Mr_D: mr[2 * D:],
        U_Mh_S: mh[:D], U_Mh_rM: mh[D:2 * D], U_Mh_D: mh[2 * D:],
        U_DS: T(W_DS), U_DM: T(W_DM), U_DD: T(W_DD),
        U_gD_S: gd[:D], U_gD_M: gd[D:],
    }

    # per-phase-instance power-of-2 scale; a unit may appear in two phase
    # instances (same units both steps) -> same scale computed per unit set.
    scl = np.zeros(NPH, f)
    unit_scale = {}
    for (step, ph), units in PH_UNITS.items():
        m = max(np.abs(raw[u]).max() for u in units)
        s = f(2.0 ** np.floor(np.log2(120.0 / m)))
        scl[ph_slot(step, ph)] = 1.0 / s
        for u in units:
            if u in unit_scale:
                assert unit_scale[u] == s, (u, unit_scale[u], s)
            unit_scale[u] = s

    u8 = [None] * NUNITS
    u16 = [None] * NUNITS
    for u, w in raw.items():
        ws = w * unit_scale[u]
        u8[u] = _pack_unit(ws, E4M3)
        u16[u] = _pack_unit(ws, BF16)
    wu8 = np.stack(u8)
    wu16 = np.stack(u16)

    bias = np.stack([np.asarray(b, f) for b in (b_gS, b_Mz, b_Mr, b_Mh, b_gD)])
    bias = bias.reshape(NBIAS, MC, 128)
    bias = np.ascontiguousarray(np.transpose(bias, (2, 0, 1)))

    scl_arr = np.ascontiguousarray(
        np.broadcast_to(scl[None, :], (128, NPH)).astype(f))
    rs = np.full((128, 1), np.asarray(residual_scale, f), dtype=f)

    hpT = np.asarray(h_prev, f).T      # [D, B]
    hnT = np.asarray(h_next, f).T

    in_maps = []
    for c in range(NCORES):
        sl = slice(c * BC, (c + 1) * BC)
        hp_c = np.ascontiguousarray(hpT[:, sl])
        hn_c = np.ascontiguousarray(hnT[:, sl])
        in_maps.append({
            "hp16": _pack_acts(hp_c, BF16),
            "hp8": _pack_acts(hp_c, E4M3),
            "hn16": _pack_acts(hn_c, BF16),
            "hn8": _pack_acts(hn_c, E4M3),
            "wu8": wu8,
            "wu16": wu16,
            "bias": bias,
            "scl": scl_arr,
            "rs": rs,
        })
    return in_maps


def _unpack_output(results):
    blocks = []
    for c in range(NCORES):
        a = results[c]["out"]                       # [NHALF, MC, 128, H]
        a = np.transpose(a, (1, 2, 0, 3)).reshape(D, BC)
        blocks.append(a)
    outT = np.concatenate(blocks, axis=1)           # [D, B]
    return np.ascontiguousarray(outT.T)


def run(trace=False, tmpdir=None, trace_kwargs=None, **inputs):
    nc = _build()
    in_maps = _pack_inputs(**inputs)
    res = run_bass_kernel_spmd(
        nc, in_maps, core_ids=list(range(NCORES)),
        trace=trace, tmpdir=tmpdir, **(trace_kwargs or {}))
    return _unpack_output(res.results), res


def kernel(**inputs):
    import os
    os.environ["BASS_NEVER_TRACE"] = "1"
    try:
        out, _ = run(**inputs)
    finally:
        os.environ.pop("BASS_NEVER_TRACE", None)
    return out
